# revision 3
# baseline (speedup 1.0000x reference)
"""Trainium2 Bass kernel for the DCE (dynamic contrast-enhanced MRI) forward model.

Pipeline (per frame f of 50):
    CA   = k1[f] * x_c[0] + k2[f] * x_c[1]            (complex, 320x320)
    w    = E1 * exp(c*CA)                              (complex exp)
    sig  = A + B / (1 - q*w)                           (rewritten signal model)
    out  = G @ sig @ G                                 (fftshifted ortho 2D DFT)

where G = P F P is the symmetric shifted DFT matrix, so ifft2c(sig) = G sig G.
The gather over time indices is folded into per-frame scalars k1/k2 on the host.
The constant A is dropped on device and added back on the host as a single
DC pixel (G @ (A*ones) @ G = 320*A at [160,160]).

Sharding: 50 frames -> 8 cores x 7 frame slots (SPMD, padded with zero coefs).

Device kernel structure (v2 — tuned against the TimelineSim cost model):
  - per frame: b/a = ratio-combines (DVE), cos/sin/exp (ACT, one table's
    worth of work each), mp = p*cbp (Pool), dnegn = p*sbn (DVE),
    |d|^2/|B| via two ACT Squares + Pool add, then sig written straight
    into the S_virtual layout with fused DIVIDES (stt (mp+1)/d2 on DVE,
    tails on Pool) — no reciprocal op, one fewer chain stage.
  - frame 0 is processed in three row-group chunks aligned with the
    matmul K-tiles, each with its own input-DMA slice, so the first
    pass-1 matmuls start ~11us in instead of waiting ~25us for a
    monolithic pointwise chain.  gv (DFT matrix) DMAs are interleaved
    between chunk DMAs so they land just before the matmuls need them.
  - two chained complex matmul passes (fp32r, full-rate at N=320) with
    "virtual-K" stacking: 640 contraction rows (320 re + 320 im) packed
    into five full K=128 tiles.  Pass1: P1 = S.T @ G ; Pass2:
    out = P1.T @ G = G S G.
  - the 12 PSUM->SBUF evictions per frame are spread DVE/ACT/Pool
    (3/4/5) to keep every engine under the PE roofline; the last
    frame's output is evicted + DMA'd per (comp, m-tile) to shrink the
    drain tail.
"""

import sys

import numpy as np

for _p in ("/opt/trn_rl_repo", "/root/.axon_site/_ro/trn_rl_repo"):
    if _p not in sys.path:
        sys.path.insert(0, _p)

import concourse.bass as bass
import concourse.mybir as mybir
from concourse import bacc
from concourse.bass_utils import run_bass_kernel_spmd
from concourse.tile import TileContext

H = W = 320
NS = 50          # frames
NCORES = 8
FPC = 7          # frame slots per core (8*7 = 56 >= 50)
P = 128
F32 = mybir.dt.float32
F32R = mybir.dt.float32r
MSIZES = ((0, 128), (128, 128), (256, 64))   # m-tiles of the 320 output rows

# ---- signal model constants (mirrors reference fp32 arithmetic) ----
_f32 = np.float32
FA = _f32(10.0 * np.pi / 180.0)
TR = _f32(0.00487)
R1 = _f32(1.0)
R1CA = _f32(4.3)
SIG0 = _f32(100.0)
E1 = np.exp(-TR * R1, dtype=np.float32)
Q = np.cos(FA, dtype=np.float32)
M0 = SIG0 * (1 - Q * E1) / (np.sin(FA) * (1 - E1))
M0T = M0 * np.sin(FA)
MST = M0T * (1 - E1) / (1 - E1 * Q)
OFFS = SIG0 - MST
C = -TR * R1CA
CONST_A = float(M0T / Q + OFFS)
CONST_B = float(-M0T * (1 - Q) / Q)
BIAS_LNQE1 = float(np.log(Q * E1))

_PROGRAM = None


def _build_program():
    """Build the single SPMD NeuronCore program (same for all 8 cores)."""
    nc = bacc.Bacc("TRN2", target_bir_lowering=False, debug=False,
                   num_devices=NCORES)
    AF = mybir.ActivationFunctionType
    OP = mybir.AluOpType

    xs_d = nc.dram_tensor("xs", [4, P, 3, W], F32, kind="ExternalInput")
    gv_d = nc.dram_tensor("gv", [2, P, 5, W], F32R, kind="ExternalInput")
    coef_d = nc.dram_tensor("coef", [P, FPC, 2], F32, kind="ExternalInput")
    out_d = nc.dram_tensor("out", [FPC, 2, 3, P, W], F32, kind="ExternalOutput")

    sqscale = float(np.sqrt(-1.0 / CONST_B))

    with TileContext(nc) as tc:
        with (
            tc.tile_pool(name="const", bufs=1) as cpool,
            tc.tile_pool(name="work", bufs=1) as wpool,
            tc.tile_pool(name="trig", bufs=4) as tpool,
            tc.tile_pool(name="sv", bufs=3) as svpool,
            tc.tile_pool(name="av", bufs=4) as avpool,
            tc.tile_pool(name="ost", bufs=2) as opool,
            tc.tile_pool(name="psum", bufs=8, space="PSUM") as pspool,
        ):
            # DMA order tuned for the frame-0 pipeline: coef first (gates
            # every chain), then per-row-group x chunks with the two gv
            # planes interleaved so they arrive just before pass-1 needs
            # them.  imag planes (1,3) lead: the sin path starts earliest.
            coef_sb = cpool.tile([P, FPC, 2], F32)
            nc.sync.dma_start(coef_sb[:], coef_d[:])
            xs_sb = cpool.tile([P, 4, 3, W], F32)
            gv_sb = cpool.tile([P, 2, 5, W], F32R)
            for pl in (1, 3, 0, 2):
                nc.sync.dma_start(xs_sb[:, pl, 0], xs_d[pl, :, 0])
            nc.sync.dma_start(gv_sb[:, 0], gv_d[0])
            for pl in (1, 3, 0, 2):
                nc.sync.dma_start(xs_sb[:, pl, 1], xs_d[pl, :, 1])
            nc.sync.dma_start(gv_sb[:, 1], gv_d[1])
            for pl in (1, 3, 0, 2):
                nc.sync.dma_start(xs_sb[:, pl, 2], xs_d[pl, :, 2])

            bias_exp = cpool.tile([P, 1], F32)
            nc.vector.memset(bias_exp[:], BIAS_LNQE1)
            bias_sin = cpool.tile([P, 1], F32)
            nc.vector.memset(bias_sin[:], float(-np.pi / 2))
            bias_nsq = cpool.tile([P, 1], F32)
            nc.vector.memset(bias_nsq[:], float(-np.sqrt(1.0 / -CONST_B)))

            x0r = xs_sb[:, 0]
            x0i = xs_sb[:, 1]
            x1r = xs_sb[:, 2]
            x1i = xs_sb[:, 3]

            def pointwise_chunk(f, tiles, g, psl):
                """Emit the signal-model chain for row-group g of frame f.

                psl: partition slice (0:128 for g=0,1; 0:64 for g=2).
                Writes sv[:, g] / sv[:, 2+g] (re/im) for g<2, and the
                kt4 mixed tail for g=2.
                """
                ck1 = coef_sb[psl, f, 0:1]
                rat = coef_sb[psl, f, 1:2]
                b, a_, cbp, sbn, p_, mp, dnegn, sq1, sq2, d2n, sv = tiles
                gs = (psl, slice(g, g + 1))
                nc.vector.scalar_tensor_tensor(b[gs], x1i[gs], rat, x0i[gs],
                                               OP.mult, OP.add)
                nc.scalar.activation(cbp[gs], b[gs], AF.Sin,
                                     bias=bias_sin[psl], scale=ck1)
                nc.scalar.activation(sbn[gs], b[gs], AF.Sin, scale=ck1)
                nc.vector.scalar_tensor_tensor(a_[gs], x1r[gs], rat, x0r[gs],
                                               OP.mult, OP.add)
                nc.scalar.activation(p_[gs], a_[gs], AF.Exp,
                                     bias=bias_exp[psl], scale=ck1)
                # mp = -q*wr ; dnegn = +q*wi  (w = E1CA); sv holds -S,
                # the sign is restored on the host (DFT is linear)
                nc.gpsimd.tensor_tensor(mp[gs], p_[gs], cbp[gs], OP.mult)
                nc.vector.tensor_tensor(dnegn[gs], p_[gs], sbn[gs], OP.mult)
                # d2n = ((1+mp)^2 + dnegn^2)/|B| ; the (1+mp) shift and
                # 1/|B| scale are folded into the ACT Square scale/bias
                nc.scalar.activation(sq1[gs], mp[gs], AF.Square,
                                     scale=-sqscale, bias=bias_nsq[psl])
                nc.scalar.activation(sq2[gs], dnegn[gs], AF.Square,
                                     scale=sqscale)
                nc.gpsimd.tensor_tensor(d2n[gs], sq1[gs], sq2[gs], OP.add)
                # fused divides: sig_re - A = (1+mp)/d2n ; sig_im = dnegn/d2n
                if g < 2:
                    nc.vector.scalar_tensor_tensor(sv[:, g], mp[gs[0], g],
                                                   1.0, d2n[gs[0], g],
                                                   OP.add, OP.divide)
                    nc.vector.tensor_tensor(sv[:, 2 + g], dnegn[gs[0], g],
                                            d2n[gs[0], g], OP.divide)
                else:
                    nc.gpsimd.scalar_tensor_tensor(sv[0:64, 4], mp[0:64, 2],
                                                   1.0, d2n[0:64, 2],
                                                   OP.add, OP.divide)
                    tail = wpool.tile([P, W], F32R, name=f"tail_{f}",
                                      tag="tail", bufs=2)
                    nc.gpsimd.tensor_tensor(tail[0:64], dnegn[0:64, 2],
                                            d2n[0:64, 2], OP.divide)
                    nc.sync.dma_start(sv[64:128, 4], tail[0:64])

            def frame_tiles(f):
                mk = lambda nm, bufs, shape=(P, 3, W), dt=F32: wpool.tile(
                    list(shape), dt, name=f"{nm}_{f}", tag=nm, bufs=bufs)
                return (mk("b", 2), mk("a", 2),
                        tpool.tile([P, 3, W], F32, name=f"cbp_{f}", tag="cbp"),
                        tpool.tile([P, 3, W], F32, name=f"sbn_{f}", tag="sbn"),
                        mk("p", 2), mk("mp", 3), mk("dnegn", 2),
                        mk("sq1", 2), mk("sq2", 2), mk("d2n", 3),
                        svpool.tile([P, 5, W], F32R, name=f"sv_{f}", tag="sv"))

            def pass1(f, sv):
                """P1 = S.T @ G (complex via virtual-K).  kt emission order
                (0,2,1,3,4) matches chunk readiness for frame 0."""
                p1 = []
                for mt, (m0, msz) in enumerate(MSIZES):
                    pre = pspool.tile([P, W], F32, name=f"p1re_{f}_{mt}", tag="ps")
                    pim = pspool.tile([P, W], F32, name=f"p1im_{f}_{mt}", tag="ps")
                    for kt in (0, 2, 1, 3, 4):
                        nc.tensor.matmul(pre[:msz], sv[:, kt, m0:m0 + msz],
                                         gv_sb[:, 0, kt], start=kt == 0,
                                         stop=kt == 4)
                    for kt in (0, 2, 1, 3, 4):
                        nc.tensor.matmul(pim[:msz], sv[:, kt, m0:m0 + msz],
                                         gv_sb[:, 1, kt], start=kt == 0,
                                         stop=kt == 4)
                    p1.append((pre, pim))
                return p1

            def assemble_av(f, p1):
                """A_virtual from P1 PSUM tiles; evictions spread ACT/DVE/Pool."""
                av = avpool.tile([P, 5, W], F32R, name=f"av_{f}", tag="av")
                nc.scalar.copy(av[:, 0], p1[0][0][:])
                nc.scalar.copy(av[:, 1], p1[1][0][:])
                nc.scalar.copy(av[0:64, 4], p1[2][0][0:64])
                nc.vector.tensor_copy(av[:, 2], p1[0][1][:])
                nc.gpsimd.tensor_copy(av[:, 3], p1[1][1][:])
                tail2 = wpool.tile([P, W], F32R, name=f"tail2_{f}",
                                   tag="tail2", bufs=2)
                nc.gpsimd.tensor_copy(tail2[0:64], p1[2][1][0:64])
                nc.sync.dma_start(av[64:128, 4], tail2[0:64])
                return av

            def pass2(f, av, fine_tail):
                """out = P1.T @ G -> staging -> HBM.  fine_tail: evict + DMA
                per (comp, m-tile) so the last frame drains fast."""
                ost = opool.tile([P, 2, 3, W], F32, name=f"ost_{f}", tag="ost")
                for mt, (m0, msz) in enumerate(MSIZES):
                    qre = pspool.tile([P, W], F32, name=f"p2re_{f}_{mt}", tag="ps")
                    qim = pspool.tile([P, W], F32, name=f"p2im_{f}_{mt}", tag="ps")
                    for kt in range(5):
                        nc.tensor.matmul(qre[:msz], av[:, kt, m0:m0 + msz],
                                         gv_sb[:, 0, kt], start=kt == 0,
                                         stop=kt == 4)
                    for kt in range(5):
                        nc.tensor.matmul(qim[:msz], av[:, kt, m0:m0 + msz],
                                         gv_sb[:, 1, kt], start=kt == 0,
                                         stop=kt == 4)
                    if mt == 0:
                        nc.scalar.copy(ost[:msz, 0, mt], qre[:msz])
                        nc.vector.tensor_copy(ost[:msz, 1, mt], qim[:msz])
                    elif mt == 1:
                        nc.gpsimd.tensor_copy(ost[:msz, 0, mt], qre[:msz])
                        nc.vector.tensor_copy(ost[:msz, 1, mt], qim[:msz])
                    else:
                        nc.gpsimd.tensor_copy(ost[:msz, 0, mt], qre[:msz])
                        nc.gpsimd.tensor_copy(ost[:msz, 1, mt], qim[:msz])
                    if fine_tail:
                        for comp in range(2):
                            if mt < 2:
                                nc.sync.dma_start(out_d[f, comp, mt],
                                                  ost[:, comp, mt])
                            else:
                                nc.sync.dma_start(out_d[f, comp, 2, 0:64],
                                                  ost[0:64, comp, 2])
                if not fine_tail:
                    for comp in range(2):
                        # mt0+mt1 merged into one DMA (HBM AP reordered);
                        # mt2 separate (only 64 valid partitions)
                        nc.sync.dma_start(
                            out_d[f, comp, 0:2].rearrange("t p w -> p t w"),
                            ost[:, comp, 0:2])
                        nc.sync.dma_start(out_d[f, comp, 2, 0:64],
                                          ost[0:64, comp, 2])

            # ---- frame 0: chunked chain at high priority (short lead-in) ----
            with tc.high_priority():
                t0 = frame_tiles(0)
                pointwise_chunk(0, t0, 0, slice(0, P))
                pointwise_chunk(0, t0, 1, slice(0, P))
                pointwise_chunk(0, t0, 2, slice(0, 64))
                p1_0 = pass1(0, t0[-1])
                av_0 = assemble_av(0, p1_0)
                pass2(0, av_0, False)

            # ---- frames 1..FPC-1: monolithic chains ----
            def pointwise_full(f, tiles):
                ck1 = coef_sb[:, f, 0:1]
                rat = coef_sb[:, f, 1:2]
                b, a_, cbp, sbn, p_, mp, dnegn, sq1, sq2, d2n, sv = tiles
                nc.vector.scalar_tensor_tensor(b[:], x1i, rat, x0i,
                                               OP.mult, OP.add)
                nc.scalar.activation(cbp[:], b[:], AF.Sin,
                                     bias=bias_sin[:], scale=ck1)
                nc.scalar.activation(sbn[:], b[:], AF.Sin, scale=ck1)
                nc.vector.scalar_tensor_tensor(a_[:], x1r, rat, x0r,
                                               OP.mult, OP.add)
                nc.scalar.activation(p_[:], a_[:], AF.Exp,
                                     bias=bias_exp[:], scale=ck1)
                nc.gpsimd.tensor_tensor(mp[:], p_[:], cbp[:], OP.mult)
                nc.vector.tensor_tensor(dnegn[:], p_[:], sbn[:], OP.mult)
                nc.scalar.activation(sq1[:], mp[:], AF.Square,
                                     scale=-sqscale, bias=bias_nsq[:])
                nc.scalar.activation(sq2[:], dnegn[:], AF.Square,
                                     scale=sqscale)
                nc.gpsimd.tensor_tensor(d2n[:], sq1[:], sq2[:], OP.add)
                nc.vector.scalar_tensor_tensor(sv[:, 0:2], mp[:, 0:2], 1.0,
                                               d2n[:, 0:2], OP.add, OP.divide)
                nc.vector.tensor_tensor(sv[:, 2:4], dnegn[:, 0:2],
                                        d2n[:, 0:2], OP.divide)
                nc.gpsimd.scalar_tensor_tensor(sv[0:64, 4], mp[0:64, 2], 1.0,
                                               d2n[0:64, 2], OP.add, OP.divide)
                tail = wpool.tile([P, W], F32R, name=f"tail_{f}",
                                  tag="tail", bufs=2)
                nc.gpsimd.tensor_tensor(tail[0:64], dnegn[0:64, 2],
                                        d2n[0:64, 2], OP.divide)
                nc.sync.dma_start(sv[64:128, 4], tail[0:64])

            for f in range(1, FPC):
                tf = frame_tiles(f)
                pointwise_full(f, tf)
                p1 = pass1(f, tf[-1])
                av = assemble_av(f, p1)
                pass2(f, av, f == FPC - 1)

    nc.compile()
    return nc


def _get_program():
    global _PROGRAM
    if _PROGRAM is None:
        _PROGRAM = _build_program()
    return _PROGRAM


def _pack_rows(plane):
    """[320, W] -> [P, 3, W] with row r stored at [r % 128, r // 128]."""
    padded = np.zeros((3 * P, W), np.float32)
    padded[:H] = plane
    return np.ascontiguousarray(padded.reshape(3, P, W).transpose(1, 0, 2))


def _host_inputs(x, aifci, t_samp, sample_time):
    x = np.asarray(x, np.float32)
    aifci = np.asarray(aifci, np.float32)
    t_samp = np.asarray(t_samp, np.float32)
    st = np.asarray(sample_time, np.float32)

    k_time = np.cumsum(aifci, dtype=np.float32) * np.float32(0.1)
    idx = np.argmin(np.abs(t_samp[None, :] - st[:, None]), axis=1)
    k1 = k_time[idx]
    k2 = aifci[idx]

    xs = np.stack([
        _pack_rows(x[0, :, :, 0]),
        _pack_rows(x[0, :, :, 1]),
        _pack_rows(x[1, :, :, 0]),
        _pack_rows(x[1, :, :, 1]),
    ])

    kk = np.arange(H, dtype=np.float64)
    g = np.exp(-2j * np.pi * np.outer(kk + 160, kk + 160) / H) / np.sqrt(H)
    gr = g.real.astype(np.float32)
    gi = g.imag.astype(np.float32)
    # virtual-K row layout: [re 0:256 | im 0:256 | re 256:320 ; im 256:320]
    gvre = np.concatenate([gr[0:256], -gi[0:256], gr[256:320], -gi[256:320]])
    gvim = np.concatenate([gi[0:256], gr[0:256], gi[256:320], gr[256:320]])
    gv = np.stack([
        np.ascontiguousarray(gvre.reshape(5, P, W).transpose(1, 0, 2)),
        np.ascontiguousarray(gvim.reshape(5, P, W).transpose(1, 0, 2)),
    ])

    # per-frame scalars, pre-multiplied by c (exp/sin take them as `scale`)
    coefs = np.zeros((NCORES, P, FPC, 2), np.float32)
    for c in range(NCORES):
        for s in range(FPC):
            fidx = c * FPC + s
            if fidx < NS:
                ck1 = np.float32(C) * k1[fidx]
                ck2 = np.float32(C) * k2[fidx]
                coefs[c, :, s, 0] = ck1
                coefs[c, :, s, 1] = ck2 / ck1 if ck1 != 0 else np.float32(0)

    return xs, gv, coefs


def _unpack_outputs(results):
    out = np.empty((NS, H, W), np.complex64)
    dc = np.float32(CONST_A * H)   # G @ (A*ones) @ G == 320*A at [160,160]
    for c in range(NCORES):
        o = np.asarray(results[c]["out"])  # [FPC, 2, 3, P, W]
        for s in range(FPC):
            fidx = c * FPC + s
            if fidx >= NS:
                break
            re = -o[s, 0].reshape(3 * P, W)[:H]
            im = -o[s, 1].reshape(3 * P, W)[:H]
            re[160, 160] += dc
            out[fidx] = re + 1j * im
    return out


def kernel(x, aifci, t_samp, sample_time):
    xs, gv, coefs = _host_inputs(x, aifci, t_samp, sample_time)
    nc = _get_program()
    in_maps = [{"xs": xs, "gv": gv, "coef": coefs[c]} for c in range(NCORES)]
    try:
        res = run_bass_kernel_spmd(nc, in_maps, list(range(NCORES)))
    except Exception:
        # a previous process can leave a NeuronCore wedged; one retry after a
        # short pause recovers it (the runtime resets the exec unit)
        import time
        time.sleep(5)
        res = run_bass_kernel_spmd(nc, in_maps, list(range(NCORES)))
    return _unpack_outputs(res.results)


# revision 7
# speedup vs baseline: 1.0360x; 1.0360x over previous
"""Trainium2 Bass kernel for the DCE (dynamic contrast-enhanced MRI) forward model.

Pipeline (per frame f of 50):
    CA   = k1[f] * x_c[0] + k2[f] * x_c[1]            (complex, 320x320)
    w    = E1 * exp(c*CA)                              (complex exp)
    sig  = A + B / (1 - q*w)                           (rewritten signal model)
    out  = G @ sig @ G                                 (fftshifted ortho 2D DFT)

where G = P F P is the symmetric shifted DFT matrix, so ifft2c(sig) = G sig G.
The gather over time indices is folded into per-frame scalars k1/k2 on the host.
The constant A is dropped on device and added back on the host as a single
DC pixel (G @ (A*ones) @ G = 320*A at [160,160]).

Sharding: 50 frames -> 8 cores x 7 frame slots (SPMD, padded with zero coefs).

Device kernel structure (v3 — tuned against the TimelineSim cost model):
  - signal model per frame: b/a ratio-combines (DVE), cos/sin via phased
    Sin and exp (ACT), mp = p*cbp (Pool), dnegn = p*sbn (DVE), then
    |d|^2/|B| = 1/|B| + 2mp/|B| + (p/sqrt|B|)^2  — one ACT Square off p
    plus a tensor_scalar + add, which is one ACT op and one dependency
    stage cheaper than squaring both components.  sig is written into
    the S_virtual layout with fused DIVIDES ((mp+1)/d2, dnegn/d2) — no
    reciprocal pass.
  - ACT function-table discipline: Sin and Exp live in different table
    sets (1.283us per reload in the cost model), Square/Copy are in
    every set.  Frames run in groups ((0,1),(1,4),(4,7)); within a
    group all Sins run first, then all Exp/Square work, pinned by dep
    edges -> 6 table loads total while frame 0's chain stays short.
  - frame 0 is processed in three row-group chunks aligned with the
    matmul K-tiles, each with its own input-DMA slice, so pass-1
    matmuls start ~11us in instead of ~25us (monolithic chain).  gv
    DMAs are interleaved between chunk DMAs to land just before use.
  - two chained complex matmul passes (fp32r, full-rate at N=320) with
    "virtual-K" stacking: 640 contraction rows (320 re + 320 im) packed
    into five full K=128 tiles.  Pass1: P1 = S.T @ G ; Pass2:
    out = P1.T @ G = G S G.  PE emission is software-pipelined:
    pass1(f+1) is queued between pass1(f)'s eviction and pass2(f), so
    the tensor engine never idles (idling also drops it to a slower
    pstate for 3us in the cost model).
  - the 12 PSUM->SBUF evictions per frame are spread ACT/DVE/Pool
    (4/3/5) to keep every engine under the PE roofline; the last
    frame's output is evicted + DMA'd per (comp, m-tile) to shrink the
    drain tail.
"""

import sys

import numpy as np

for _p in ("/opt/trn_rl_repo", "/root/.axon_site/_ro/trn_rl_repo"):
    if _p not in sys.path:
        sys.path.insert(0, _p)

import concourse.bass as bass
import concourse.mybir as mybir
from concourse import bacc
from concourse.bass_utils import run_bass_kernel_spmd
from concourse.tile import TileContext

H = W = 320
NS = 50          # frames
NCORES = 8
FPC = 7          # frame slots per core (8*7 = 56 >= 50)
P = 128
F32 = mybir.dt.float32
F32R = mybir.dt.float32r
MSIZES = ((0, 128), (128, 128), (256, 64))   # m-tiles of the 320 output rows
GROUPS = ((0, 1), (1, 4), (4, FPC))          # act-table phase groups

# ---- signal model constants (mirrors reference fp32 arithmetic) ----
_f32 = np.float32
FA = _f32(10.0 * np.pi / 180.0)
TR = _f32(0.00487)
R1 = _f32(1.0)
R1CA = _f32(4.3)
SIG0 = _f32(100.0)
E1 = np.exp(-TR * R1, dtype=np.float32)
Q = np.cos(FA, dtype=np.float32)
M0 = SIG0 * (1 - Q * E1) / (np.sin(FA) * (1 - E1))
M0T = M0 * np.sin(FA)
MST = M0T * (1 - E1) / (1 - E1 * Q)
OFFS = SIG0 - MST
C = -TR * R1CA
CONST_A = float(M0T / Q + OFFS)
CONST_B = float(-M0T * (1 - Q) / Q)
BIAS_LNQE1 = float(np.log(Q * E1))

_PROGRAM = None


def _build_program():
    """Build the single SPMD NeuronCore program (same for all 8 cores)."""
    nc = bacc.Bacc("TRN2", target_bir_lowering=False, debug=False,
                   num_devices=NCORES)
    AF = mybir.ActivationFunctionType
    OP = mybir.AluOpType

    xs_d = nc.dram_tensor("xs", [4, P, 3, W], F32, kind="ExternalInput")
    gv_d = nc.dram_tensor("gv", [2, P, 5, W], F32R, kind="ExternalInput")
    coef_d = nc.dram_tensor("coef", [P, FPC, 2], F32, kind="ExternalInput")
    out_d = nc.dram_tensor("out", [FPC, 2, 3, P, W], F32, kind="ExternalOutput")

    sqscale = float(np.sqrt(-1.0 / CONST_B))   # 1/sqrt(|B|)
    twob = float(-2.0 / CONST_B)               # 2/|B|
    invb = float(-1.0 / CONST_B)               # 1/|B|

    from concourse.tile_rust import add_dep_helper

    with TileContext(nc) as tc:
        with (
            tc.tile_pool(name="const", bufs=1) as cpool,
            tc.tile_pool(name="work", bufs=1) as wpool,
            tc.tile_pool(name="trig", bufs=4) as tpool,
            tc.tile_pool(name="sv", bufs=3) as svpool,
            tc.tile_pool(name="av", bufs=4) as avpool,
            tc.tile_pool(name="ost", bufs=2) as opool,
            tc.tile_pool(name="psum", bufs=8, space="PSUM") as pspool,
        ):
            # DMA order tuned for the frame-0 pipeline: coef first (gates
            # every chain), then per-row-group x chunks with the two gv
            # planes interleaved so they arrive just before pass-1 needs
            # them.  imag planes (1,3) lead: the sin path starts earliest.
            coef_sb = cpool.tile([P, FPC, 2], F32)
            nc.sync.dma_start(coef_sb[:], coef_d[:])
            xs_sb = cpool.tile([P, 4, 3, W], F32)
            gv_sb = cpool.tile([P, 2, 5, W], F32R)
            for pl in (1, 3, 0, 2):
                nc.sync.dma_start(xs_sb[:, pl, 0], xs_d[pl, :, 0])
            nc.sync.dma_start(gv_sb[:, 0], gv_d[0])
            for pl in (1, 3, 0, 2):
                nc.sync.dma_start(xs_sb[:, pl, 1], xs_d[pl, :, 1])
            nc.sync.dma_start(gv_sb[:, 1], gv_d[1])
            for pl in (1, 3, 0, 2):
                nc.sync.dma_start(xs_sb[:, pl, 2], xs_d[pl, :, 2])

            bias_exp = cpool.tile([P, 1], F32)
            nc.vector.memset(bias_exp[:], BIAS_LNQE1)
            bias_sin = cpool.tile([P, 1], F32)
            nc.vector.memset(bias_sin[:], float(-np.pi / 2))

            x0r = xs_sb[:, 0]
            x0i = xs_sb[:, 1]
            x1r = xs_sb[:, 2]
            x1i = xs_sb[:, 3]

            CHUNKS = ((0, slice(0, P)), (1, slice(0, P)), (2, slice(0, 64)))

            def frame_tiles(f):
                mk = lambda nm, bufs: wpool.tile(
                    [P, 3, W], F32, name=f"{nm}_{f}", tag=nm, bufs=bufs)
                return {
                    "b": mk("b", 4), "a": mk("a", 4),
                    "cbp": tpool.tile([P, 3, W], F32, name=f"cbp_{f}", tag="cbp"),
                    "sbn": tpool.tile([P, 3, W], F32, name=f"sbn_{f}", tag="sbn"),
                    "p": mk("p", 2), "mp": mk("mp", 3), "dn": mk("dn", 2),
                    "sq": mk("sq", 2), "t2": mk("t2", 2), "d2": mk("d2", 3),
                    "sv": svpool.tile([P, 5, W], F32R, name=f"sv_{f}", tag="sv"),
                }

            # ---- sin phase: b + the two Sin lookups (cos via -pi/2 bias) ----
            # chunked=True emits per row-group (frame 0's low-latency path);
            # otherwise one monolithic op per tensor.
            def sin_part(f, t, chunked):
                first = last = None
                parts = CHUNKS if chunked else ((slice(0, 3), slice(0, P)),)
                for g, psl in parts:
                    ck1 = coef_sb[psl, f, 0:1]
                    rat = coef_sb[psl, f, 1:2]
                    gs = (psl, slice(g, g + 1)) if chunked else (psl, g)
                    nc.vector.scalar_tensor_tensor(t["b"][gs], x1i[gs], rat,
                                                   x0i[gs], OP.mult, OP.add)
                    i1 = nc.scalar.activation(t["cbp"][gs], t["b"][gs], AF.Sin,
                                              bias=bias_sin[psl], scale=ck1)
                    i2 = nc.scalar.activation(t["sbn"][gs], t["b"][gs], AF.Sin,
                                              scale=ck1)
                    first = first or i1
                    last = i2
                return first, last

            # ---- exp phase: exp, square, and the rest of the chain ----
            def exp_part(f, t, chunked):
                first = None
                parts = CHUNKS if chunked else ((slice(0, 3), slice(0, P)),)
                for g, psl in parts:
                    ck1 = coef_sb[psl, f, 0:1]
                    rat = coef_sb[psl, f, 1:2]
                    gs = (psl, slice(g, g + 1)) if chunked else (psl, g)
                    nc.vector.scalar_tensor_tensor(t["a"][gs], x1r[gs], rat,
                                                   x0r[gs], OP.mult, OP.add)
                    # p = q*E1*exp(ck1*a) ; mp = -q*wr ; dn = +q*wi (sv
                    # holds -S, the sign is restored on the host)
                    i1 = nc.scalar.activation(t["p"][gs], t["a"][gs], AF.Exp,
                                              bias=bias_exp[psl], scale=ck1)
                    first = first or i1
                    nc.gpsimd.tensor_tensor(t["mp"][gs], t["p"][gs],
                                            t["cbp"][gs], OP.mult)
                    nc.vector.tensor_tensor(t["dn"][gs], t["p"][gs],
                                            t["sbn"][gs], OP.mult)
                    # |d|^2/|B| = 1/|B| + 2*mp/|B| + (p/sqrt|B|)^2
                    nc.scalar.activation(t["sq"][gs], t["p"][gs], AF.Square,
                                         scale=sqscale)
                    nc.vector.tensor_scalar(t["t2"][gs], t["mp"][gs], twob,
                                            invb, OP.mult, OP.add)
                    nc.gpsimd.tensor_tensor(t["d2"][gs], t["t2"][gs],
                                            t["sq"][gs], OP.add)
                    # fused divides into the S_virtual layout
                    if chunked and g < 2:
                        nc.vector.scalar_tensor_tensor(
                            t["sv"][:, g], t["mp"][psl, g], 1.0,
                            t["d2"][psl, g], OP.add, OP.divide)
                        nc.vector.tensor_tensor(
                            t["sv"][:, 2 + g], t["dn"][psl, g],
                            t["d2"][psl, g], OP.divide)
                    elif chunked:
                        sv_tail(f, t)
                if not chunked:
                    nc.vector.scalar_tensor_tensor(
                        t["sv"][:, 0:2], t["mp"][:, 0:2], 1.0,
                        t["d2"][:, 0:2], OP.add, OP.divide)
                    nc.vector.tensor_tensor(t["sv"][:, 2:4], t["dn"][:, 0:2],
                                            t["d2"][:, 0:2], OP.divide)
                    sv_tail(f, t)
                return first

            def sv_tail(f, t):
                nc.gpsimd.scalar_tensor_tensor(
                    t["sv"][0:64, 4], t["mp"][0:64, 2], 1.0,
                    t["d2"][0:64, 2], OP.add, OP.divide)
                tail = wpool.tile([P, W], F32R, name=f"tail_{f}",
                                  tag="tail", bufs=2)
                nc.gpsimd.tensor_tensor(tail[0:64], t["dn"][0:64, 2],
                                        t["d2"][0:64, 2], OP.divide)
                nc.sync.dma_start(t["sv"][64:128, 4], tail[0:64])

            def pass1(f, sv):
                """P1 = S.T @ G (complex via virtual-K).  kt emission order
                (0,2,1,3,4) matches chunk readiness for frame 0."""
                p1 = []
                for mt, (m0, msz) in enumerate(MSIZES):
                    pre = pspool.tile([P, W], F32, name=f"p1re_{f}_{mt}", tag="ps")
                    pim = pspool.tile([P, W], F32, name=f"p1im_{f}_{mt}", tag="ps")
                    for kt in (0, 2, 1, 3, 4):
                        nc.tensor.matmul(pre[:msz], sv[:, kt, m0:m0 + msz],
                                         gv_sb[:, 0, kt], start=kt == 0,
                                         stop=kt == 4)
                    for kt in (0, 2, 1, 3, 4):
                        nc.tensor.matmul(pim[:msz], sv[:, kt, m0:m0 + msz],
                                         gv_sb[:, 1, kt], start=kt == 0,
                                         stop=kt == 4)
                    p1.append((pre, pim))
                return p1

            def assemble_av(f, p1):
                """A_virtual from P1 PSUM tiles; evictions spread ACT/DVE/Pool."""
                av = avpool.tile([P, 5, W], F32R, name=f"av_{f}", tag="av")
                nc.scalar.copy(av[:, 0], p1[0][0][:])
                nc.scalar.copy(av[:, 1], p1[1][0][:])
                nc.gpsimd.tensor_copy(av[0:64, 4], p1[2][0][0:64])
                nc.vector.tensor_copy(av[:, 2], p1[0][1][:])
                nc.gpsimd.tensor_copy(av[:, 3], p1[1][1][:])
                tail2 = wpool.tile([P, W], F32R, name=f"tail2_{f}",
                                   tag="tail2", bufs=2)
                nc.gpsimd.tensor_copy(tail2[0:64], p1[2][1][0:64])
                nc.sync.dma_start(av[64:128, 4], tail2[0:64])
                return av

            def pass2(f, av, fine_tail):
                """out = P1.T @ G -> staging -> HBM.  fine_tail: evict + DMA
                per (comp, m-tile) so the last frame drains fast."""
                ost = opool.tile([P, 2, 3, W], F32, name=f"ost_{f}", tag="ost")
                for mt, (m0, msz) in enumerate(MSIZES):
                    qre = pspool.tile([P, W], F32, name=f"p2re_{f}_{mt}", tag="ps")
                    qim = pspool.tile([P, W], F32, name=f"p2im_{f}_{mt}", tag="ps")
                    for kt in range(5):
                        nc.tensor.matmul(qre[:msz], av[:, kt, m0:m0 + msz],
                                         gv_sb[:, 0, kt], start=kt == 0,
                                         stop=kt == 4)
                    for kt in range(5):
                        nc.tensor.matmul(qim[:msz], av[:, kt, m0:m0 + msz],
                                         gv_sb[:, 1, kt], start=kt == 0,
                                         stop=kt == 4)
                    if mt == 0:
                        nc.scalar.copy(ost[:msz, 0, mt], qre[:msz])
                        nc.vector.tensor_copy(ost[:msz, 1, mt], qim[:msz])
                    elif mt == 1:
                        nc.scalar.copy(ost[:msz, 0, mt], qre[:msz])
                        nc.vector.tensor_copy(ost[:msz, 1, mt], qim[:msz])
                    else:
                        nc.gpsimd.tensor_copy(ost[:msz, 0, mt], qre[:msz])
                        nc.gpsimd.tensor_copy(ost[:msz, 1, mt], qim[:msz])
                    if fine_tail:
                        for comp in range(2):
                            if mt < 2:
                                nc.sync.dma_start(out_d[f, comp, mt],
                                                  ost[:, comp, mt])
                            else:
                                nc.sync.dma_start(out_d[f, comp, 2, 0:64],
                                                  ost[0:64, comp, 2])
                if not fine_tail:
                    for comp in range(2):
                        # mt0+mt1 merged into one DMA (HBM AP reordered);
                        # mt2 separate (only 64 valid partitions)
                        nc.sync.dma_start(
                            out_d[f, comp, 0:2].rearrange("t p w -> p t w"),
                            ost[:, comp, 0:2])
                        nc.sync.dma_start(out_d[f, comp, 2, 0:64],
                                          ost[0:64, comp, 2])

            # ---- emission: act-table phase groups + software-pipelined PE ----
            tiles = {}
            p1s = {}
            avs = {}
            prev_exp_first = {}
            prev_phase_last = None   # last exp-phase ACT op of prev group

            for gi, (f0, f1) in enumerate(GROUPS):
                hp = tc.high_priority() if gi == 0 else None
                if hp:
                    hp.__enter__()
                sin_last = None
                for f in range(f0, f1):
                    tiles[f] = frame_tiles(f)
                    s_first, s_last = sin_part(f, tiles[f], chunked=f == 0)
                    if prev_phase_last is not None:
                        add_dep_helper(s_first.ins, prev_phase_last.ins,
                                       reason="act-table phase order")
                    sin_last = s_last
                for f in range(f0, f1):
                    e_first = exp_part(f, tiles[f], chunked=f == 0)
                    add_dep_helper(e_first.ins, sin_last.ins,
                                   reason="act-table phase order")
                    prev_phase_last = e_first
                    # PE pipeline: av(f-1) evictions, pass1(f), pass2(f-1)
                    if f - 1 in p1s:
                        avs[f - 1] = assemble_av(f - 1, p1s[f - 1])
                    p1s[f] = pass1(f, tiles[f]["sv"])
                    if f - 1 in avs:
                        pass2(f - 1, avs[f - 1], False)
                if hp:
                    hp.__exit__(None, None, None)

            avs[FPC - 1] = assemble_av(FPC - 1, p1s[FPC - 1])
            pass2(FPC - 1, avs[FPC - 1], True)

    nc.compile()
    return nc


def _get_program():
    global _PROGRAM
    if _PROGRAM is None:
        _PROGRAM = _build_program()
    return _PROGRAM


def _pack_rows(plane):
    """[320, W] -> [P, 3, W] with row r stored at [r % 128, r // 128]."""
    padded = np.zeros((3 * P, W), np.float32)
    padded[:H] = plane
    return np.ascontiguousarray(padded.reshape(3, P, W).transpose(1, 0, 2))


def _host_inputs(x, aifci, t_samp, sample_time):
    x = np.asarray(x, np.float32)
    aifci = np.asarray(aifci, np.float32)
    t_samp = np.asarray(t_samp, np.float32)
    st = np.asarray(sample_time, np.float32)

    k_time = np.cumsum(aifci, dtype=np.float32) * np.float32(0.1)
    idx = np.argmin(np.abs(t_samp[None, :] - st[:, None]), axis=1)
    k1 = k_time[idx]
    k2 = aifci[idx]

    xs = np.stack([
        _pack_rows(x[0, :, :, 0]),
        _pack_rows(x[0, :, :, 1]),
        _pack_rows(x[1, :, :, 0]),
        _pack_rows(x[1, :, :, 1]),
    ])

    kk = np.arange(H, dtype=np.float64)
    g = np.exp(-2j * np.pi * np.outer(kk + 160, kk + 160) / H) / np.sqrt(H)
    gr = g.real.astype(np.float32)
    gi = g.imag.astype(np.float32)
    # virtual-K row layout: [re 0:256 | im 0:256 | re 256:320 ; im 256:320]
    gvre = np.concatenate([gr[0:256], -gi[0:256], gr[256:320], -gi[256:320]])
    gvim = np.concatenate([gi[0:256], gr[0:256], gi[256:320], gr[256:320]])
    gv = np.stack([
        np.ascontiguousarray(gvre.reshape(5, P, W).transpose(1, 0, 2)),
        np.ascontiguousarray(gvim.reshape(5, P, W).transpose(1, 0, 2)),
    ])

    # per-frame scalars, pre-multiplied by c (exp/sin take them as `scale`)
    coefs = np.zeros((NCORES, P, FPC, 2), np.float32)
    for c in range(NCORES):
        for s in range(FPC):
            fidx = c * FPC + s
            if fidx < NS:
                ck1 = np.float32(C) * k1[fidx]
                ck2 = np.float32(C) * k2[fidx]
                coefs[c, :, s, 0] = ck1
                coefs[c, :, s, 1] = ck2 / ck1 if ck1 != 0 else np.float32(0)

    return xs, gv, coefs


def _unpack_outputs(results):
    out = np.empty((NS, H, W), np.complex64)
    dc = np.float32(CONST_A * H)   # G @ (A*ones) @ G == 320*A at [160,160]
    for c in range(NCORES):
        o = np.asarray(results[c]["out"])  # [FPC, 2, 3, P, W]
        for s in range(FPC):
            fidx = c * FPC + s
            if fidx >= NS:
                break
            re = -o[s, 0].reshape(3 * P, W)[:H]
            im = -o[s, 1].reshape(3 * P, W)[:H]
            re[160, 160] += dc
            out[fidx] = re + 1j * im
    return out


def kernel(x, aifci, t_samp, sample_time):
    xs, gv, coefs = _host_inputs(x, aifci, t_samp, sample_time)
    nc = _get_program()
    in_maps = [{"xs": xs, "gv": gv, "coef": coefs[c]} for c in range(NCORES)]
    try:
        res = run_bass_kernel_spmd(nc, in_maps, list(range(NCORES)))
    except Exception:
        # a previous process can leave a NeuronCore wedged; one retry after a
        # short pause recovers it (the runtime resets the exec unit)
        import time
        time.sleep(5)
        res = run_bass_kernel_spmd(nc, in_maps, list(range(NCORES)))
    return _unpack_outputs(res.results)


# revision 10
# speedup vs baseline: 1.0626x; 1.0257x over previous
"""Trainium2 Bass kernel for the DCE (dynamic contrast-enhanced MRI) forward model.

Pipeline (per frame f of 50):
    CA   = k1[f] * x_c[0] + k2[f] * x_c[1]            (complex, 320x320)
    w    = E1 * exp(c*CA)                              (complex exp)
    sig  = A + B / (1 - q*w)                           (rewritten signal model)
    out  = G @ sig @ G                                 (fftshifted ortho 2D DFT)

where G = P F P is the symmetric shifted DFT matrix, so ifft2c(sig) = G sig G.
The gather over time indices is folded into per-frame scalars k1/k2 on the host.
The constant A is dropped on device and added back on the host as a single
DC pixel (G @ (A*ones) @ G = 320*A at [160,160]).

Sharding: 50 frames -> 8 cores x 7 frame slots (SPMD, padded with zero coefs).

Device kernel structure (v3 — tuned against the TimelineSim cost model):
  - signal model per frame: b/a ratio-combines (DVE), cos/sin via phased
    Sin and exp (ACT), mp = p*cbp (Pool), dnegn = p*sbn (DVE), then
    |d|^2/|B| = 1/|B| + 2mp/|B| + (p/sqrt|B|)^2  — one ACT Square off p
    plus a tensor_scalar + add, which is one ACT op and one dependency
    stage cheaper than squaring both components.  sig is written into
    the S_virtual layout with fused DIVIDES ((mp+1)/d2, dnegn/d2) — no
    reciprocal pass.
  - ACT function-table discipline: Sin and Exp live in different table
    sets (1.283us per reload in the cost model), Square/Copy are in
    every set.  Frames run in groups ((0,1),(1,4),(4,7)); within a
    group all Sins run first, then all Exp/Square work, pinned by dep
    edges -> 6 table loads total while frame 0's chain stays short.
  - frame 0 is processed in three row-group chunks aligned with the
    matmul K-tiles, each with its own input-DMA slice, so pass-1
    matmuls start ~11us in instead of ~25us (monolithic chain).  gv
    DMAs are interleaved between chunk DMAs to land just before use.
  - two chained complex matmul passes (fp32r, full-rate at N=320) with
    "virtual-K" stacking: 640 contraction rows (320 re + 320 im) packed
    into five full K=128 tiles.  Pass1: P1 = S.T @ G ; Pass2:
    out = P1.T @ G = G S G.  PE emission is software-pipelined:
    pass1(f+1) is queued between pass1(f)'s eviction and pass2(f), so
    the tensor engine never idles (idling also drops it to a slower
    pstate for 3us in the cost model).
  - the 12 PSUM->SBUF evictions per frame are spread ACT/DVE/Pool
    (4/3/5) to keep every engine under the PE roofline; the last
    frame's output is evicted + DMA'd per (comp, m-tile) to shrink the
    drain tail.
"""

import sys

import numpy as np

for _p in ("/opt/trn_rl_repo", "/root/.axon_site/_ro/trn_rl_repo"):
    if _p not in sys.path:
        sys.path.insert(0, _p)

import concourse.bass as bass
import concourse.mybir as mybir
from concourse import bacc
from concourse.bass_utils import run_bass_kernel_spmd
from concourse.tile import TileContext

H = W = 320
NS = 50          # frames
NCORES = 8
FPC = 7          # frame slots per core (8*7 = 56 >= 50)
P = 128
F32 = mybir.dt.float32
F32R = mybir.dt.float32r
MSIZES = ((0, 128), (128, 128), (256, 64))   # m-tiles of the 320 output rows
GROUPS = ((0, 1), (1, 3), (3, FPC))          # act-table phase groups

# ---- signal model constants (mirrors reference fp32 arithmetic) ----
_f32 = np.float32
FA = _f32(10.0 * np.pi / 180.0)
TR = _f32(0.00487)
R1 = _f32(1.0)
R1CA = _f32(4.3)
SIG0 = _f32(100.0)
E1 = np.exp(-TR * R1, dtype=np.float32)
Q = np.cos(FA, dtype=np.float32)
M0 = SIG0 * (1 - Q * E1) / (np.sin(FA) * (1 - E1))
M0T = M0 * np.sin(FA)
MST = M0T * (1 - E1) / (1 - E1 * Q)
OFFS = SIG0 - MST
C = -TR * R1CA
CONST_A = float(M0T / Q + OFFS)
CONST_B = float(-M0T * (1 - Q) / Q)
BIAS_LNQE1 = float(np.log(Q * E1))

_PROGRAM = None


def _build_program():
    """Build the single SPMD NeuronCore program (same for all 8 cores)."""
    nc = bacc.Bacc("TRN2", target_bir_lowering=False, debug=False,
                   num_devices=NCORES)
    AF = mybir.ActivationFunctionType
    OP = mybir.AluOpType

    xs_d = nc.dram_tensor("xs", [4, P, 3, W], F32, kind="ExternalInput")
    gv_d = nc.dram_tensor("gv", [2, P, 5, W], F32R, kind="ExternalInput")
    coef_d = nc.dram_tensor("coef", [P, FPC, 2], F32, kind="ExternalInput")
    out_d = nc.dram_tensor("out", [FPC, 2, 3, P, W], F32, kind="ExternalOutput")

    sqscale = float(np.sqrt(-1.0 / CONST_B))   # 1/sqrt(|B|)
    twob = float(-2.0 / CONST_B)               # 2/|B|
    invb = float(-1.0 / CONST_B)               # 1/|B|

    from concourse.tile_rust import add_dep_helper

    with TileContext(nc) as tc:
        with (
            tc.tile_pool(name="const", bufs=1) as cpool,
            tc.tile_pool(name="work", bufs=1) as wpool,
            tc.tile_pool(name="trig", bufs=4) as tpool,
            tc.tile_pool(name="sv", bufs=3) as svpool,
            tc.tile_pool(name="av", bufs=4) as avpool,
            tc.tile_pool(name="ost", bufs=2) as opool,
            tc.tile_pool(name="psum", bufs=8, space="PSUM") as pspool,
        ):
            # DMA order tuned for the frame-0 pipeline: coef first (gates
            # every chain), then per-row-group x chunks with the two gv
            # planes interleaved so they arrive just before pass-1 needs
            # them.  imag planes (1,3) lead: the sin path starts earliest.
            coef_sb = cpool.tile([P, FPC, 2], F32)
            nc.sync.dma_start(coef_sb[:], coef_d[:])
            xs_sb = cpool.tile([P, 4, 3, W], F32)
            gv_sb = cpool.tile([P, 2, 5, W], F32R)
            for pl in (1, 3, 0, 2):
                nc.sync.dma_start(xs_sb[:, pl], xs_d[pl])
            nc.sync.dma_start(gv_sb[:, 0], gv_d[0])
            nc.sync.dma_start(gv_sb[:, 1], gv_d[1])

            bias_exp = cpool.tile([P, 1], F32)
            nc.vector.memset(bias_exp[:], BIAS_LNQE1)
            bias_sin = cpool.tile([P, 1], F32)
            nc.vector.memset(bias_sin[:], float(-np.pi / 2))

            x0r = xs_sb[:, 0]
            x0i = xs_sb[:, 1]
            x1r = xs_sb[:, 2]
            x1i = xs_sb[:, 3]

            def frame_tiles(f):
                mk = lambda nm, bufs: wpool.tile(
                    [P, 3, W], F32, name=f"{nm}_{f}", tag=nm, bufs=bufs)
                return {
                    "b": mk("b", 4), "a": mk("a", 4),
                    "cbp": tpool.tile([P, 3, W], F32, name=f"cbp_{f}", tag="cbp"),
                    "sbn": tpool.tile([P, 3, W], F32, name=f"sbn_{f}", tag="sbn"),
                    "p": mk("p", 2), "mp": mk("mp", 3), "dn": mk("dn", 2),
                    "sq": mk("sq", 2), "t2": mk("t2", 2), "d2": mk("d2", 3),
                    "sv": svpool.tile([P, 5, W], F32R, name=f"sv_{f}", tag="sv"),
                }

            # ---- sin phase: b + the two Sin lookups (cos via -pi/2 bias) ----
            def sin_part(f, t):
                ck1 = coef_sb[:, f, 0:1]
                rat = coef_sb[:, f, 1:2]
                nc.vector.scalar_tensor_tensor(t["b"][:], x1i, rat, x0i,
                                               OP.mult, OP.add)
                i1 = nc.scalar.activation(t["cbp"][:], t["b"][:], AF.Sin,
                                          bias=bias_sin[:], scale=ck1)
                i2 = nc.scalar.activation(t["sbn"][:], t["b"][:], AF.Sin,
                                          scale=ck1)
                return i1, i2

            # ---- exp phase: exp, square, and the rest of the chain ----
            def exp_part(f, t):
                ck1 = coef_sb[:, f, 0:1]
                rat = coef_sb[:, f, 1:2]
                nc.vector.scalar_tensor_tensor(t["a"][:], x1r, rat, x0r,
                                               OP.mult, OP.add)
                # p = q*E1*exp(ck1*a) ; mp = -q*wr ; dn = +q*wi (sv holds
                # -S, the sign is restored on the host)
                i1 = nc.scalar.activation(t["p"][:], t["a"][:], AF.Exp,
                                          bias=bias_exp[:], scale=ck1)
                nc.vector.tensor_tensor(t["mp"][:], t["p"][:], t["cbp"][:],
                                        OP.mult)
                nc.vector.tensor_tensor(t["dn"][:], t["p"][:], t["sbn"][:],
                                        OP.mult)
                # |d|^2/|B| = 1/|B| + 2*mp/|B| + (p/sqrt|B|)^2 ; the affine
                # term rides the ACT Copy scale/bias (Copy needs no table)
                nc.scalar.activation(t["sq"][:], t["p"][:], AF.Square,
                                     scale=sqscale)
                i4 = nc.scalar.activation(t["t2"][:], t["mp"][:], AF.Copy,
                                          bias=invb, scale=twob)
                nc.vector.tensor_tensor(t["d2"][:], t["t2"][:], t["sq"][:],
                                        OP.add)
                # fused divides into the S_virtual layout
                nc.vector.scalar_tensor_tensor(
                    t["sv"][:, 0:2], t["mp"][:, 0:2], 1.0,
                    t["d2"][:, 0:2], OP.add, OP.divide)
                nc.vector.tensor_tensor(t["sv"][:, 2:4], t["dn"][:, 0:2],
                                        t["d2"][:, 0:2], OP.divide)
                nc.gpsimd.scalar_tensor_tensor(
                    t["sv"][0:64, 4], t["mp"][0:64, 2], 1.0,
                    t["d2"][0:64, 2], OP.add, OP.divide)
                tail = wpool.tile([P, W], F32R, name=f"tail_{f}",
                                  tag="tail", bufs=2)
                nc.gpsimd.tensor_tensor(tail[0:64], t["dn"][0:64, 2],
                                        t["d2"][0:64, 2], OP.divide)
                nc.sync.dma_start(t["sv"][64:128, 4], tail[0:64])
                return i1, i4

            def pass1(f, sv):
                """P1 = S.T @ G (complex via virtual-K).  kt emission order
                (0,2,1,3,4) matches chunk readiness for frame 0."""
                p1 = []
                for mt, (m0, msz) in enumerate(MSIZES):
                    pre = pspool.tile([P, W], F32, name=f"p1re_{f}_{mt}", tag="ps")
                    pim = pspool.tile([P, W], F32, name=f"p1im_{f}_{mt}", tag="ps")
                    for kt in (0, 2, 1, 3, 4):
                        nc.tensor.matmul(pre[:msz], sv[:, kt, m0:m0 + msz],
                                         gv_sb[:, 0, kt], start=kt == 0,
                                         stop=kt == 4)
                    for kt in (0, 2, 1, 3, 4):
                        nc.tensor.matmul(pim[:msz], sv[:, kt, m0:m0 + msz],
                                         gv_sb[:, 1, kt], start=kt == 0,
                                         stop=kt == 4)
                    p1.append((pre, pim))
                return p1

            def assemble_av(f, p1):
                """A_virtual from P1 PSUM tiles; evictions spread ACT/Pool."""
                av = avpool.tile([P, 5, W], F32R, name=f"av_{f}", tag="av")
                nc.scalar.copy(av[:, 0], p1[0][0][:])
                nc.scalar.copy(av[:, 1], p1[1][0][:])
                nc.gpsimd.tensor_copy(av[0:64, 4], p1[2][0][0:64])
                nc.gpsimd.tensor_copy(av[:, 2], p1[0][1][:])
                nc.gpsimd.tensor_copy(av[:, 3], p1[1][1][:])
                tail2 = wpool.tile([P, W], F32R, name=f"tail2_{f}",
                                   tag="tail2", bufs=2)
                nc.gpsimd.tensor_copy(tail2[0:64], p1[2][1][0:64])
                nc.sync.dma_start(av[64:128, 4], tail2[0:64])
                return av

            def pass2(f, av, fine_tail):
                """out = P1.T @ G -> staging -> HBM.  fine_tail: evict + DMA
                per (comp, m-tile) so the last frame drains fast."""
                ost = opool.tile([P, 2, 3, W], F32, name=f"ost_{f}", tag="ost")
                for mt, (m0, msz) in enumerate(MSIZES):
                    qre = pspool.tile([P, W], F32, name=f"p2re_{f}_{mt}", tag="ps")
                    qim = pspool.tile([P, W], F32, name=f"p2im_{f}_{mt}", tag="ps")
                    for kt in range(5):
                        nc.tensor.matmul(qre[:msz], av[:, kt, m0:m0 + msz],
                                         gv_sb[:, 0, kt], start=kt == 0,
                                         stop=kt == 4)
                    for kt in range(5):
                        nc.tensor.matmul(qim[:msz], av[:, kt, m0:m0 + msz],
                                         gv_sb[:, 1, kt], start=kt == 0,
                                         stop=kt == 4)
                    if mt < 2:
                        nc.scalar.copy(ost[:msz, 0, mt], qre[:msz])
                        nc.gpsimd.tensor_copy(ost[:msz, 1, mt], qim[:msz])
                    else:
                        nc.gpsimd.tensor_copy(ost[:msz, 0, mt], qre[:msz])
                        nc.gpsimd.tensor_copy(ost[:msz, 1, mt], qim[:msz])
                    if fine_tail:
                        for comp in range(2):
                            if mt < 2:
                                nc.sync.dma_start(out_d[f, comp, mt],
                                                  ost[:, comp, mt])
                            else:
                                nc.sync.dma_start(out_d[f, comp, 2, 0:64],
                                                  ost[0:64, comp, 2])
                if not fine_tail:
                    for comp in range(2):
                        # mt0+mt1 merged into one DMA (HBM AP reordered);
                        # mt2 separate (only 64 valid partitions)
                        nc.sync.dma_start(
                            out_d[f, comp, 0:2].rearrange("t p w -> p t w"),
                            ost[:, comp, 0:2])
                        nc.sync.dma_start(out_d[f, comp, 2, 0:64],
                                          ost[0:64, comp, 2])

            # ---- emission: act-table phase groups + software-pipelined PE ----
            tiles = {}
            p1s = {}
            avs = {}
            prev_phase_lasts = []   # exp-phase ACT tails of prev group

            for gi, (f0, f1) in enumerate(GROUPS):
                hp = tc.high_priority() if gi == 0 else None
                if hp:
                    hp.__enter__()
                sin_last = None
                for f in range(f0, f1):
                    tiles[f] = frame_tiles(f)
                    s_first, s_last = sin_part(f, tiles[f])
                    for pl_ in prev_phase_lasts:
                        add_dep_helper(s_first.ins, pl_.ins,
                                       reason="act-table phase order")
                    sin_last = s_last
                prev_phase_lasts = []
                for f in range(f0, f1):
                    e_first, e_last = exp_part(f, tiles[f])
                    add_dep_helper(e_first.ins, sin_last.ins,
                                   reason="act-table phase order")
                    prev_phase_lasts.append(e_last)
                    # PE pipeline: av(f-1) evictions, pass1(f), pass2(f-1)
                    if f - 1 in p1s:
                        avs[f - 1] = assemble_av(f - 1, p1s[f - 1])
                    p1s[f] = pass1(f, tiles[f]["sv"])
                    if f - 1 in avs:
                        pass2(f - 1, avs[f - 1], False)
                if hp:
                    hp.__exit__(None, None, None)

            avs[FPC - 1] = assemble_av(FPC - 1, p1s[FPC - 1])
            pass2(FPC - 1, avs[FPC - 1], True)

    nc.compile()
    return nc


def _get_program():
    global _PROGRAM
    if _PROGRAM is None:
        _PROGRAM = _build_program()
    return _PROGRAM


def _pack_rows(plane):
    """[320, W] -> [P, 3, W] with row r stored at [r % 128, r // 128]."""
    padded = np.zeros((3 * P, W), np.float32)
    padded[:H] = plane
    return np.ascontiguousarray(padded.reshape(3, P, W).transpose(1, 0, 2))


def _host_inputs(x, aifci, t_samp, sample_time):
    x = np.asarray(x, np.float32)
    aifci = np.asarray(aifci, np.float32)
    t_samp = np.asarray(t_samp, np.float32)
    st = np.asarray(sample_time, np.float32)

    k_time = np.cumsum(aifci, dtype=np.float32) * np.float32(0.1)
    idx = np.argmin(np.abs(t_samp[None, :] - st[:, None]), axis=1)
    k1 = k_time[idx]
    k2 = aifci[idx]

    xs = np.stack([
        _pack_rows(x[0, :, :, 0]),
        _pack_rows(x[0, :, :, 1]),
        _pack_rows(x[1, :, :, 0]),
        _pack_rows(x[1, :, :, 1]),
    ])

    kk = np.arange(H, dtype=np.float64)
    g = np.exp(-2j * np.pi * np.outer(kk + 160, kk + 160) / H) / np.sqrt(H)
    gr = g.real.astype(np.float32)
    gi = g.imag.astype(np.float32)
    # virtual-K row layout: [re 0:256 | im 0:256 | re 256:320 ; im 256:320]
    gvre = np.concatenate([gr[0:256], -gi[0:256], gr[256:320], -gi[256:320]])
    gvim = np.concatenate([gi[0:256], gr[0:256], gi[256:320], gr[256:320]])
    gv = np.stack([
        np.ascontiguousarray(gvre.reshape(5, P, W).transpose(1, 0, 2)),
        np.ascontiguousarray(gvim.reshape(5, P, W).transpose(1, 0, 2)),
    ])

    # per-frame scalars, pre-multiplied by c (exp/sin take them as `scale`)
    coefs = np.zeros((NCORES, P, FPC, 2), np.float32)
    for c in range(NCORES):
        for s in range(FPC):
            fidx = c * FPC + s
            if fidx < NS:
                ck1 = np.float32(C) * k1[fidx]
                ck2 = np.float32(C) * k2[fidx]
                coefs[c, :, s, 0] = ck1
                coefs[c, :, s, 1] = ck2 / ck1 if ck1 != 0 else np.float32(0)

    return xs, gv, coefs


def _unpack_outputs(results):
    out = np.empty((NS, H, W), np.complex64)
    dc = np.float32(CONST_A * H)   # G @ (A*ones) @ G == 320*A at [160,160]
    for c in range(NCORES):
        o = np.asarray(results[c]["out"])  # [FPC, 2, 3, P, W]
        for s in range(FPC):
            fidx = c * FPC + s
            if fidx >= NS:
                break
            re = -o[s, 0].reshape(3 * P, W)[:H]
            im = -o[s, 1].reshape(3 * P, W)[:H]
            re[160, 160] += dc
            out[fidx] = re + 1j * im
    return out


def kernel(x, aifci, t_samp, sample_time):
    xs, gv, coefs = _host_inputs(x, aifci, t_samp, sample_time)
    nc = _get_program()
    in_maps = [{"xs": xs, "gv": gv, "coef": coefs[c]} for c in range(NCORES)]
    try:
        res = run_bass_kernel_spmd(nc, in_maps, list(range(NCORES)))
    except Exception:
        # a previous process can leave a NeuronCore wedged; one retry after a
        # short pause recovers it (the runtime resets the exec unit)
        import time
        time.sleep(5)
        res = run_bass_kernel_spmd(nc, in_maps, list(range(NCORES)))
    return _unpack_outputs(res.results)


# revision 11
# speedup vs baseline: 1.1698x; 1.1009x over previous
"""Trainium2 Bass kernel for the DCE (dynamic contrast-enhanced MRI) forward model.

Pipeline (per frame f of 50):
    CA   = k1[f] * x_c[0] + k2[f] * x_c[1]            (complex, 320x320)
    w    = E1 * exp(c*CA)                              (complex exp)
    sig  = A + B / (1 - q*w)                           (rewritten signal model)
    out  = G @ sig @ G                                 (fftshifted ortho 2D DFT)

where G = P F P is the symmetric shifted DFT matrix, so ifft2c(sig) = G sig G.
The gather over time indices is folded into per-frame scalars k1/k2 on the host.
The constant A is dropped on device and added back on the host as a single
DC pixel (G @ (A*ones) @ G = 320*A at [160,160]).

Sharding: 50 frames -> 8 cores x 7 frame slots (SPMD, padded with zero coefs).

Device kernel structure (v3 — tuned against the TimelineSim cost model):
  - signal model per frame: b/a ratio-combines (DVE), cos/sin via phased
    Sin and exp (ACT), mp = p*cbp (Pool), dnegn = p*sbn (DVE), then
    |d|^2/|B| = 1/|B| + 2mp/|B| + (p/sqrt|B|)^2  — one ACT Square off p
    plus a tensor_scalar + add, which is one ACT op and one dependency
    stage cheaper than squaring both components.  sig is written into
    the S_virtual layout with fused DIVIDES ((mp+1)/d2, dnegn/d2) — no
    reciprocal pass.
  - ACT function-table discipline: Sin and Exp live in different table
    sets (1.283us per reload in the cost model), Square/Copy are in
    every set.  Frames run in groups ((0,1),(1,4),(4,7)); within a
    group all Sins run first, then all Exp/Square work, pinned by dep
    edges -> 6 table loads total while frame 0's chain stays short.
  - frame 0 is processed in three row-group chunks aligned with the
    matmul K-tiles, each with its own input-DMA slice, so pass-1
    matmuls start ~11us in instead of ~25us (monolithic chain).  gv
    DMAs are interleaved between chunk DMAs to land just before use.
  - two chained complex matmul passes (fp32r, full-rate at N=320) with
    "virtual-K" stacking: 640 contraction rows (320 re + 320 im) packed
    into five full K=128 tiles.  Pass1: P1 = S.T @ G ; Pass2:
    out = P1.T @ G = G S G.  PE emission is software-pipelined:
    pass1(f+1) is queued between pass1(f)'s eviction and pass2(f), so
    the tensor engine never idles (idling also drops it to a slower
    pstate for 3us in the cost model).
  - the 12 PSUM->SBUF evictions per frame are spread ACT/DVE/Pool
    (4/3/5) to keep every engine under the PE roofline; the last
    frame's output is evicted + DMA'd per (comp, m-tile) to shrink the
    drain tail.
"""

import sys

import numpy as np

for _p in ("/opt/trn_rl_repo", "/root/.axon_site/_ro/trn_rl_repo"):
    if _p not in sys.path:
        sys.path.insert(0, _p)

import concourse.bass as bass
import concourse.mybir as mybir
from concourse import bacc
from concourse.bass_utils import run_bass_kernel_spmd
from concourse.tile import TileContext

H = W = 320
NS = 50          # frames
NCORES = 8
FPC = 7          # frame slots per core (8*7 = 56 >= 50)
P = 128
F32 = mybir.dt.float32
F32R = mybir.dt.float32r
MSIZES = ((0, 128), (128, 128), (256, 64))   # m-tiles of the 320 output rows
GROUPS = ((0, 1), (1, 3), (3, FPC))          # act-table phase groups

# ---- signal model constants (mirrors reference fp32 arithmetic) ----
_f32 = np.float32
FA = _f32(10.0 * np.pi / 180.0)
TR = _f32(0.00487)
R1 = _f32(1.0)
R1CA = _f32(4.3)
SIG0 = _f32(100.0)
E1 = np.exp(-TR * R1, dtype=np.float32)
Q = np.cos(FA, dtype=np.float32)
M0 = SIG0 * (1 - Q * E1) / (np.sin(FA) * (1 - E1))
M0T = M0 * np.sin(FA)
MST = M0T * (1 - E1) / (1 - E1 * Q)
OFFS = SIG0 - MST
C = -TR * R1CA
CONST_A = float(M0T / Q + OFFS)
CONST_B = float(-M0T * (1 - Q) / Q)
BIAS_LNQE1 = float(np.log(Q * E1))

_PROGRAM = None


def _build_program():
    """Build the single SPMD NeuronCore program (same for all 8 cores)."""
    nc = bacc.Bacc("TRN2", target_bir_lowering=False, debug=False,
                   num_devices=NCORES)
    AF = mybir.ActivationFunctionType
    OP = mybir.AluOpType

    xs_d = nc.dram_tensor("xs", [4, P, 3, W], F32, kind="ExternalInput")
    gv_d = nc.dram_tensor("gv", [2, P, 5, W], F32R, kind="ExternalInput")
    coef_d = nc.dram_tensor("coef", [P, FPC, 2], F32, kind="ExternalInput")
    out_d = nc.dram_tensor("out", [FPC, 2, 3, P, W], F32, kind="ExternalOutput")

    sqscale = float(np.sqrt(-1.0 / CONST_B))   # 1/sqrt(|B|)
    twob = float(-2.0 / CONST_B)               # 2/|B|
    invb = float(-1.0 / CONST_B)               # 1/|B|

    from concourse.tile_rust import add_dep_helper

    with TileContext(nc) as tc:
        with (
            tc.tile_pool(name="const", bufs=1) as cpool,
            tc.tile_pool(name="work", bufs=1) as wpool,
            tc.tile_pool(name="trig", bufs=4) as tpool,
            tc.tile_pool(name="sv", bufs=3) as svpool,
            tc.tile_pool(name="av", bufs=4) as avpool,
            tc.tile_pool(name="ost", bufs=2) as opool,
            tc.tile_pool(name="psum", bufs=8, space="PSUM") as pspool,
        ):
            # DMA order tuned for the frame-0 pipeline: coef first (gates
            # every chain), then per-row-group x chunks with the two gv
            # planes interleaved so they arrive just before pass-1 needs
            # them.  imag planes (1,3) lead: the sin path starts earliest.
            coef_sb = cpool.tile([P, FPC, 2], F32)
            nc.sync.dma_start(coef_sb[:], coef_d[:])
            xs_sb = cpool.tile([P, 4, 3, W], F32)
            gv_sb = cpool.tile([P, 2, 5, W], F32R)
            for pl in (1, 3, 0, 2):
                nc.sync.dma_start(xs_sb[:, pl], xs_d[pl])
            nc.sync.dma_start(gv_sb[:, 0], gv_d[0])
            nc.sync.dma_start(gv_sb[:, 1], gv_d[1])

            bias_exp = cpool.tile([P, 1], F32)
            nc.vector.memset(bias_exp[:], BIAS_LNQE1)
            bias_sin = cpool.tile([P, 1], F32)
            nc.vector.memset(bias_sin[:], float(-np.pi / 2))

            x0r = xs_sb[:, 0]
            x0i = xs_sb[:, 1]
            x1r = xs_sb[:, 2]
            x1i = xs_sb[:, 3]

            def frame_tiles(f):
                mk = lambda nm, bufs: wpool.tile(
                    [P, 3, W], F32, name=f"{nm}_{f}", tag=nm, bufs=bufs)
                return {
                    "b": mk("b", 4), "a": mk("a", 4),
                    "cbp": tpool.tile([P, 3, W], F32, name=f"cbp_{f}", tag="cbp"),
                    "sbn": tpool.tile([P, 3, W], F32, name=f"sbn_{f}", tag="sbn"),
                    "p": mk("p", 2), "mp": mk("mp", 3), "dn": mk("dn", 2),
                    "sq": mk("sq", 2), "t2": mk("t2", 2), "d2": mk("d2", 3),
                    "sv": svpool.tile([P, 5, W], F32R, name=f"sv_{f}", tag="sv"),
                }

            # ---- sin phase: b + the two Sin lookups (cos via -pi/2 bias) ----
            def sin_part(f, t):
                ck1 = coef_sb[:, f, 0:1]
                rat = coef_sb[:, f, 1:2]
                nc.vector.scalar_tensor_tensor(t["b"][:], x1i, rat, x0i,
                                               OP.mult, OP.add)
                i1 = nc.scalar.activation(t["cbp"][:], t["b"][:], AF.Sin,
                                          bias=bias_sin[:], scale=ck1)
                i2 = nc.scalar.activation(t["sbn"][:], t["b"][:], AF.Sin,
                                          scale=ck1)
                return i1, i2

            # ---- exp phase: exp, square, and the rest of the chain ----
            def exp_part(f, t):
                ck1 = coef_sb[:, f, 0:1]
                rat = coef_sb[:, f, 1:2]
                nc.vector.scalar_tensor_tensor(t["a"][:], x1r, rat, x0r,
                                               OP.mult, OP.add)
                # p = q*E1*exp(ck1*a) ; mp = -q*wr ; dn = +q*wi (sv holds
                # -S, the sign is restored on the host)
                i1 = nc.scalar.activation(t["p"][:], t["a"][:], AF.Exp,
                                          bias=bias_exp[:], scale=ck1)
                nc.vector.tensor_tensor(t["mp"][:], t["p"][:], t["cbp"][:],
                                        OP.mult)
                nc.vector.tensor_tensor(t["dn"][:], t["p"][:], t["sbn"][:],
                                        OP.mult)
                # |d|^2/|B| = 1/|B| + 2*mp/|B| + (p/sqrt|B|)^2 ; the affine
                # term rides the ACT Copy scale/bias (Copy needs no table)
                nc.scalar.activation(t["sq"][:], t["p"][:], AF.Square,
                                     scale=sqscale)
                i4 = nc.scalar.activation(t["t2"][:], t["mp"][:], AF.Copy,
                                          bias=invb, scale=twob)
                nc.vector.tensor_tensor(t["d2"][:], t["t2"][:], t["sq"][:],
                                        OP.add)
                # fused divides into the S_virtual layout
                nc.vector.scalar_tensor_tensor(
                    t["sv"][:, 0:2], t["mp"][:, 0:2], 1.0,
                    t["d2"][:, 0:2], OP.add, OP.divide)
                nc.vector.tensor_tensor(t["sv"][:, 2:4], t["dn"][:, 0:2],
                                        t["d2"][:, 0:2], OP.divide)
                nc.gpsimd.scalar_tensor_tensor(
                    t["sv"][0:64, 4], t["mp"][0:64, 2], 1.0,
                    t["d2"][0:64, 2], OP.add, OP.divide)
                tail = wpool.tile([P, W], F32R, name=f"tail_{f}",
                                  tag="tail", bufs=2)
                nc.gpsimd.tensor_tensor(tail[0:64], t["dn"][0:64, 2],
                                        t["d2"][0:64, 2], OP.divide)
                nc.sync.dma_start(t["sv"][64:128, 4], tail[0:64])
                return i1, i4

            def pass1(f, sv):
                """P1 = S.T @ G (complex via virtual-K).  kt emission order
                (0,2,1,3,4) matches chunk readiness for frame 0."""
                p1 = []
                for mt, (m0, msz) in enumerate(MSIZES):
                    pre = pspool.tile([P, W], F32, name=f"p1re_{f}_{mt}", tag="ps")
                    pim = pspool.tile([P, W], F32, name=f"p1im_{f}_{mt}", tag="ps")
                    for kt in (0, 2, 1, 3, 4):
                        nc.tensor.matmul(pre[:msz], sv[:, kt, m0:m0 + msz],
                                         gv_sb[:, 0, kt], start=kt == 0,
                                         stop=kt == 4)
                    for kt in (0, 2, 1, 3, 4):
                        nc.tensor.matmul(pim[:msz], sv[:, kt, m0:m0 + msz],
                                         gv_sb[:, 1, kt], start=kt == 0,
                                         stop=kt == 4)
                    p1.append((pre, pim))
                return p1

            def assemble_av(f, p1):
                """A_virtual from P1 PSUM tiles; evictions spread ACT/Pool."""
                av = avpool.tile([P, 5, W], F32R, name=f"av_{f}", tag="av")
                nc.scalar.copy(av[:, 0], p1[0][0][:])
                nc.scalar.copy(av[:, 1], p1[1][0][:])
                nc.gpsimd.tensor_copy(av[0:64, 4], p1[2][0][0:64])
                nc.gpsimd.tensor_copy(av[:, 2], p1[0][1][:])
                nc.gpsimd.tensor_copy(av[:, 3], p1[1][1][:])
                tail2 = wpool.tile([P, W], F32R, name=f"tail2_{f}",
                                   tag="tail2", bufs=2)
                nc.gpsimd.tensor_copy(tail2[0:64], p1[2][1][0:64])
                nc.sync.dma_start(av[64:128, 4], tail2[0:64])
                return av

            def pass2(f, av, fine_tail):
                """out = P1.T @ G -> staging -> HBM.  fine_tail: evict + DMA
                per (comp, m-tile) so the last frame drains fast."""
                ost = opool.tile([P, 2, 3, W], F32, name=f"ost_{f}", tag="ost")
                for mt, (m0, msz) in enumerate(MSIZES):
                    qre = pspool.tile([P, W], F32, name=f"p2re_{f}_{mt}", tag="ps")
                    qim = pspool.tile([P, W], F32, name=f"p2im_{f}_{mt}", tag="ps")
                    for kt in range(5):
                        nc.tensor.matmul(qre[:msz], av[:, kt, m0:m0 + msz],
                                         gv_sb[:, 0, kt], start=kt == 0,
                                         stop=kt == 4)
                    for kt in range(5):
                        nc.tensor.matmul(qim[:msz], av[:, kt, m0:m0 + msz],
                                         gv_sb[:, 1, kt], start=kt == 0,
                                         stop=kt == 4)
                    if mt < 2:
                        nc.scalar.copy(ost[:msz, 0, mt], qre[:msz])
                        nc.gpsimd.tensor_copy(ost[:msz, 1, mt], qim[:msz])
                    else:
                        nc.gpsimd.tensor_copy(ost[:msz, 0, mt], qre[:msz])
                        nc.gpsimd.tensor_copy(ost[:msz, 1, mt], qim[:msz])
                    if fine_tail:
                        for comp in range(2):
                            if mt < 2:
                                nc.sync.dma_start(out_d[f, comp, mt],
                                                  ost[:, comp, mt])
                            else:
                                nc.sync.dma_start(out_d[f, comp, 2, 0:64],
                                                  ost[0:64, comp, 2])
                if not fine_tail:
                    for comp in range(2):
                        # mt0+mt1 merged into one DMA (HBM AP reordered);
                        # mt2 separate (only 64 valid partitions)
                        nc.sync.dma_start(
                            out_d[f, comp, 0:2].rearrange("t p w -> p t w"),
                            ost[:, comp, 0:2])
                        nc.sync.dma_start(out_d[f, comp, 2, 0:64],
                                          ost[0:64, comp, 2])

            # ---- emission: act-table phase groups + software-pipelined PE ----
            tiles = {}
            p1s = {}
            avs = {}
            prev_phase_lasts = []   # exp-phase ACT tails of prev group

            for gi, (f0, f1) in enumerate(GROUPS):
                hp = tc.high_priority() if gi == 0 else None
                if hp:
                    hp.__enter__()
                group_sins = []
                for f in range(f0, f1):
                    tiles[f] = frame_tiles(f)
                    s_i1, s_i2 = sin_part(f, tiles[f])
                    for si in (s_i1, s_i2):
                        for pl_ in prev_phase_lasts:
                            add_dep_helper(si.ins, pl_.ins,
                                           reason="act-table phase order")
                    group_sins += [s_i1, s_i2]
                prev_phase_lasts = []
                for f in range(f0, f1):
                    e_first, e_last = exp_part(f, tiles[f])
                    for si in group_sins:
                        add_dep_helper(e_first.ins, si.ins,
                                       reason="act-table phase order")
                    prev_phase_lasts.append(e_last)
                    # PE pipeline: av(f-1) evictions, pass1(f), pass2(f-1)
                    if f - 1 in p1s:
                        avs[f - 1] = assemble_av(f - 1, p1s[f - 1])
                    p1s[f] = pass1(f, tiles[f]["sv"])
                    if f - 1 in avs:
                        pass2(f - 1, avs[f - 1], False)
                if hp:
                    hp.__exit__(None, None, None)

            avs[FPC - 1] = assemble_av(FPC - 1, p1s[FPC - 1])
            pass2(FPC - 1, avs[FPC - 1], True)

    nc.compile()
    return nc


def _get_program():
    global _PROGRAM
    if _PROGRAM is None:
        _PROGRAM = _build_program()
    return _PROGRAM


def _pack_rows(plane):
    """[320, W] -> [P, 3, W] with row r stored at [r % 128, r // 128]."""
    padded = np.zeros((3 * P, W), np.float32)
    padded[:H] = plane
    return np.ascontiguousarray(padded.reshape(3, P, W).transpose(1, 0, 2))


def _host_inputs(x, aifci, t_samp, sample_time):
    x = np.asarray(x, np.float32)
    aifci = np.asarray(aifci, np.float32)
    t_samp = np.asarray(t_samp, np.float32)
    st = np.asarray(sample_time, np.float32)

    k_time = np.cumsum(aifci, dtype=np.float32) * np.float32(0.1)
    idx = np.argmin(np.abs(t_samp[None, :] - st[:, None]), axis=1)
    k1 = k_time[idx]
    k2 = aifci[idx]

    xs = np.stack([
        _pack_rows(x[0, :, :, 0]),
        _pack_rows(x[0, :, :, 1]),
        _pack_rows(x[1, :, :, 0]),
        _pack_rows(x[1, :, :, 1]),
    ])

    kk = np.arange(H, dtype=np.float64)
    g = np.exp(-2j * np.pi * np.outer(kk + 160, kk + 160) / H) / np.sqrt(H)
    gr = g.real.astype(np.float32)
    gi = g.imag.astype(np.float32)
    # virtual-K row layout: [re 0:256 | im 0:256 | re 256:320 ; im 256:320]
    gvre = np.concatenate([gr[0:256], -gi[0:256], gr[256:320], -gi[256:320]])
    gvim = np.concatenate([gi[0:256], gr[0:256], gi[256:320], gr[256:320]])
    gv = np.stack([
        np.ascontiguousarray(gvre.reshape(5, P, W).transpose(1, 0, 2)),
        np.ascontiguousarray(gvim.reshape(5, P, W).transpose(1, 0, 2)),
    ])

    # per-frame scalars, pre-multiplied by c (exp/sin take them as `scale`)
    coefs = np.zeros((NCORES, P, FPC, 2), np.float32)
    for c in range(NCORES):
        for s in range(FPC):
            fidx = c * FPC + s
            if fidx < NS:
                ck1 = np.float32(C) * k1[fidx]
                ck2 = np.float32(C) * k2[fidx]
                coefs[c, :, s, 0] = ck1
                coefs[c, :, s, 1] = ck2 / ck1 if ck1 != 0 else np.float32(0)

    return xs, gv, coefs


def _unpack_outputs(results):
    out = np.empty((NS, H, W), np.complex64)
    dc = np.float32(CONST_A * H)   # G @ (A*ones) @ G == 320*A at [160,160]
    for c in range(NCORES):
        o = np.asarray(results[c]["out"])  # [FPC, 2, 3, P, W]
        for s in range(FPC):
            fidx = c * FPC + s
            if fidx >= NS:
                break
            re = -o[s, 0].reshape(3 * P, W)[:H]
            im = -o[s, 1].reshape(3 * P, W)[:H]
            re[160, 160] += dc
            out[fidx] = re + 1j * im
    return out


def kernel(x, aifci, t_samp, sample_time):
    xs, gv, coefs = _host_inputs(x, aifci, t_samp, sample_time)
    nc = _get_program()
    in_maps = [{"xs": xs, "gv": gv, "coef": coefs[c]} for c in range(NCORES)]
    try:
        res = run_bass_kernel_spmd(nc, in_maps, list(range(NCORES)))
    except Exception:
        # a previous process can leave a NeuronCore wedged; one retry after a
        # short pause recovers it (the runtime resets the exec unit)
        import time
        time.sleep(5)
        res = run_bass_kernel_spmd(nc, in_maps, list(range(NCORES)))
    return _unpack_outputs(res.results)
